# revision 13
# baseline (speedup 1.0000x reference)
"""Trainium2 Bass kernel for nn_ChunkedConvblock (chunked conv + local attention + LN + pool).

Reference computation per chunk of L=64 timesteps (D=512):
  ct = relu(conv1d(x^T, k=2, dilation=2, pad=1) + cb)^T     # [L, D]
  scores = (x @ ct^T) / sqrt(D); p = softmax(scores, -1)
  h = LN(p @ ct + ct) * g + b ; out = mean_t(h)             # [D]

Distribution: data-parallel over batch B=32 across 8 cores (4 rows/core).
Each core processes 256 chunks as 32 "supertiles" of 8 chunks (512 steps).

v6 strategy:
  - host ships x pre-transposed (d on partitions) AND pre-shifted for the
    two conv taps (xm = x[t-1], xp = x[t+1], zeroed at chunk bounds), fp16,
    laid out so each supertile slice is one contiguous 512KB DMA.
  - 2-stage software pipeline with FINE-GRAINED INTERLEAVED EMISSION:
    the attention block of supertile s is emitted matmul-by-matmul between
    the conv matmuls of supertile s+1, so every LDWEIGHTS-bound N=128
    matmul (ct transposes, scores, p-transposes) hides its weight load
    under a 213ns stream-bound conv matmul.
  - ACT uses only {Relu, Exp, Copy} = one activation table set -> no
    1.28us table reloads. rstd = fast-inverse-sqrt (bit trick + 2 Newton
    steps) on DVE ints.
  - layernorm gain/bias commute with the time-mean, applied after pooling.
"""

import sys

if "/opt/trn_rl_repo" not in sys.path:
    sys.path.insert(0, "/opt/trn_rl_repo")

import numpy as np

import concourse.bass as bass
from concourse.bacc import Bacc
from concourse import mybir
from concourse.tile import TileContext
from concourse.bass_utils import run_bass_kernel_spmd

F32 = mybir.dt.float32
F16 = mybir.dt.float16
I32 = mybir.dt.int32
AF = mybir.ActivationFunctionType
ALU = mybir.AluOpType

D = 512
L = 64
SUP = 512          # timesteps per supertile
CH = SUP // L      # 8 chunks per supertile
FISR_MAGIC = 0x5F3759DF


def build_program(B_loc: int, T: int, reps: int = 1):
    """Build the per-core Bass program."""
    n_sup_row = T // SUP
    n_sup = B_loc * n_sup_row
    n_chunks = B_loc * T // L

    nc = Bacc()
    xq_d = nc.declare_dram_parameter("xq", [B_loc, n_sup_row, 128, 4, SUP], F16, isOutput=False)
    xm_d = nc.declare_dram_parameter("xm", [B_loc, n_sup_row, 128, 4, SUP], F16, isOutput=False)
    xp_d = nc.declare_dram_parameter("xp", [B_loc, n_sup_row, 128, 4, SUP], F16, isOutput=False)
    wt = nc.declare_dram_parameter("wt", [128, 8, D], F16, isOutput=False)
    cb = nc.declare_dram_parameter("cb", [D], F32, isOutput=False)
    g = nc.declare_dram_parameter("g", [D], F32, isOutput=False)
    bb = nc.declare_dram_parameter("bb", [D], F32, isOutput=False)
    pw = nc.declare_dram_parameter("pw", [128, 32], F16, isOutput=False)
    ident = nc.declare_dram_parameter("ident", [128, 128], F16, isOutput=False)
    out = nc.declare_dram_parameter("out", [n_chunks, D], F32, isOutput=True)

    inv_sqrt_d = float(1.0 / np.sqrt(D))
    from contextlib import nullcontext

    with TileContext(nc) as tc, \
         tc.tile_pool(name="singles", bufs=1) as singles, \
         tc.tile_pool(name="xq", bufs=3) as xq_pool, \
         tc.tile_pool(name="xm", bufs=3) as xm_pool, \
         tc.tile_pool(name="xp", bufs=3) as xp_pool, \
         tc.tile_pool(name="ctT", bufs=4) as ctT_pool, \
         tc.tile_pool(name="ctn", bufs=3) as ctn_pool, \
         tc.tile_pool(name="soft", bufs=6) as soft_pool, \
         tc.tile_pool(name="acc", bufs=6) as acc_pool, \
         tc.tile_pool(name="stat", bufs=6) as stat_pool, \
         tc.tile_pool(name="po", bufs=2) as po_pool, \
         tc.tile_pool(name="psc", bufs=3, space="PSUM") as psc_pool, \
         tc.tile_pool(name="ps", bufs=4, space="PSUM") as ps_pool, \
         tc.tile_pool(name="psp", bufs=1, space="PSUM") as psp_pool:

        # ---- one-time constants ----
        wt_sb = singles.tile([128, 8, D], F16)
        nc.sync.dma_start(out=wt_sb, in_=wt[:, :, :])
        ident_sb = singles.tile([128, 128], F16)
        nc.sync.dma_start(out=ident_sb, in_=ident[:, :])
        cb_sb = singles.tile([128, 4], F32)
        nc.sync.dma_start(out=cb_sb, in_=cb.rearrange("(j p) -> p j", p=128))
        pw_sb = singles.tile([128, 32], F16)
        nc.sync.dma_start(out=pw_sb, in_=pw[:, :])
        g_ap = g[:]
        b_ap = bb[:]
        g8 = singles.tile([8, D], F32)
        nc.sync.dma_start(
            out=g8,
            in_=bass.AP(tensor=g_ap.tensor, offset=g_ap.offset,
                        ap=[[0, 8]] + list(g_ap.ap)),
        )
        b8 = singles.tile([8, D], F32)
        nc.sync.dma_start(
            out=b8,
            in_=bass.AP(tensor=b_ap.tensor, offset=b_ap.offset,
                        ap=[[0, 8]] + list(b_ap.ap)),
        )
        # persistent softmax tiles: off-diagonal (cross-chunk) blocks are
        # zeroed once; the per-supertile exps only rewrite the diagonals
        p_pers = []
        for i in range(4):
            p_sb = singles.tile([128, 128], F16, tag=f"p{i}")
            nc.vector.memset(p_sb[0:64, 64:128], 0.0)
            nc.vector.memset(p_sb[64:128, 0:64], 0.0)
            p_pers.append(p_sb)

        def stage_a_ops(idx):
            """Closure list: loads + conv MMs (relu attached to each 8th)."""
            row, s = divmod(idx, n_sup_row)
            xq = xq_pool.tile([128, 4, SUP], F16, tag="xq")
            xm = xm_pool.tile([128, 4, SUP], F16, tag="xm")
            xp = xp_pool.tile([128, 4, SUP], F16, tag="xp")
            ctT = ctT_pool.tile([128, 4, SUP], F16, tag="ctT")
            ops = []

            def loads():
                nc.sync.dma_start(out=xq, in_=xq_d[row, s])
                nc.sync.dma_start(out=xm, in_=xm_d[row, s])
                nc.sync.dma_start(out=xp, in_=xp_d[row, s])
            ops.append(loads)

            psc_box = {}

            def conv_mm(ob, k):
                def op():
                    if k == 0:
                        psc_box[ob] = psc_pool.tile([128, SUP], F32, tag="psc", name="psc")
                    tap, ib = divmod(k, 4)
                    xtap = xm if tap == 0 else xp
                    nc.tensor.matmul(
                        psc_box[ob],
                        wt_sb[:, tap * 4 + ib, ob * 128:(ob + 1) * 128],
                        xtap[:, ib, :],
                        start=(k == 0),
                        stop=(k == 7),
                    )
                    if k == 7:
                        nc.scalar.activation(
                            out=ctT[:, ob, :], in_=psc_box[ob], func=AF.Relu,
                            bias=cb_sb[:, ob:ob + 1], scale=1.0,
                        )
                return op

            for ob in range(4):
                for k in range(8):
                    ops.append(conv_mm(ob, k))
            return ops, (xq, ctT)

        state = {"tail": None}

        def stage_b_ops(idx, xq, ctT):
            """Closure list: prev tail, ct transposes, attention block."""
            ops = []
            if state["tail"] is not None:
                ops.extend(state["tail"])
                state["tail"] = None

            ctn = ctn_pool.tile([128, 4, D], F16, tag="ctn")
            ps2_box = {}

            def transpose_op(tb, db):
                def op():
                    if db == 0:
                        ps2_box[tb] = ps_pool.tile([128, D], F16, tag="ps", name="ps2")
                    nc.tensor.transpose(
                        ps2_box[tb][:, db * 128:(db + 1) * 128],
                        ctT[:, db, tb * 128:(tb + 1) * 128],
                        ident_sb,
                    )
                    if db == 3:
                        if tb % 2 == 0:
                            nc.scalar.copy(out=ctn[:, tb, :], in_=ps2_box[tb])
                        else:
                            nc.vector.tensor_copy(
                                out=ctn[:, tb, :], in_=ps2_box[tb])
                return op

            den_all = stat_pool.tile([128, 4], F32, tag="den")
            rec_all = stat_pool.tile([128, 4], F32, tag="rec")
            mv_all = stat_pool.tile([128, 4, 2], F32, tag="mv")
            pT_tiles = [None] * 4
            a_tiles = [None] * 4
            pss_box = {}

            def scores_mm(pr, db):
                def op():
                    if db == 0:
                        pss_box[pr] = ps_pool.tile([128, 128], F32, tag="ps", name="pss")
                    nc.tensor.matmul(
                        pss_box[pr],
                        xq[:, db, pr * 128:(pr + 1) * 128],
                        ctT[:, db, pr * 128:(pr + 1) * 128],
                        start=(db == 0), stop=(db == 3),
                    )
                    if db == 3:
                        p_sb = p_pers[pr]
                        nc.scalar.activation(
                            out=p_sb[0:64, 0:64], in_=pss_box[pr][0:64, 0:64],
                            func=AF.Exp, scale=inv_sqrt_d,
                            accum_out=den_all[0:64, pr:pr + 1],
                        )
                        nc.scalar.activation(
                            out=p_sb[64:128, 64:128],
                            in_=pss_box[pr][64:128, 64:128],
                            func=AF.Exp, scale=inv_sqrt_d,
                            accum_out=den_all[64:128, pr:pr + 1],
                        )
                        nc.vector.reciprocal(
                            rec_all[:, pr:pr + 1], den_all[:, pr:pr + 1]
                        )
                return op

            def p_transpose(pr):
                def op():
                    ps3 = ps_pool.tile([128, 128], F16, tag="ps")
                    nc.tensor.transpose(ps3, p_pers[pr], ident_sb)
                    pT_sb = soft_pool.tile([128, 128], F16, tag="pT")
                    nc.scalar.copy(out=pT_sb, in_=ps3)
                    pT_tiles[pr] = pT_sb
                return op

            def attn_mm(pr):
                def op():
                    psa = ps_pool.tile([128, D], F32, tag="ps")
                    nc.tensor.matmul(psa, pT_tiles[pr], ctn[:, pr, :])
                    a_sb = acc_pool.tile([128, D], F16, tag="a")
                    nc.vector.scalar_tensor_tensor(
                        out=a_sb, in0=psa, scalar=rec_all[:, pr:pr + 1],
                        in1=ctn[:, pr, :], op0=ALU.mult, op1=ALU.add,
                    )
                    st = stat_pool.tile([128, 6], F32, tag="st")
                    nc.vector.bn_stats(st, a_sb)
                    nc.vector.bn_aggr(mv_all[:, pr, :], st)
                    a_tiles[pr] = a_sb
                return op

            # transposes first (ctn needed by attn), then the pair pipeline
            for tb in range(4):
                for db in range(4):
                    ops.append(transpose_op(tb, db))
            sched = []
            for step in range(6):
                if step < 4:
                    sched.extend(scores_mm(step, db) for db in range(4))
                if 1 <= step <= 4:
                    sched.append(p_transpose(step - 1))
                if step >= 2:
                    sched.append(attn_mm(step - 2))
            ops.extend(sched)

            def make_tail(mv_all, a_tiles, chunk0):
                psp_box = {}

                def fisr():
                    # rstd = 1/sqrt(var + eps) via FISR + 2 Newton steps
                    veps = stat_pool.tile([128, 4], F32, tag="veps")
                    nc.vector.tensor_scalar(
                        out=veps, in0=mv_all[:, :, 1], scalar1=1e-5,
                        scalar2=None, op0=ALU.add,
                    )
                    y = stat_pool.tile([128, 4], F32, tag="y")
                    yi = y.bitcast(I32)
                    nc.vector.tensor_scalar(
                        out=yi, in0=veps.bitcast(I32), scalar1=1,
                        scalar2=None, op0=ALU.arith_shift_right,
                    )
                    nc.vector.tensor_scalar(
                        out=yi, in0=yi, scalar1=-1, scalar2=FISR_MAGIC,
                        op0=ALU.mult, op1=ALU.add,
                    )
                    sq = stat_pool.tile([128, 4], F32, tag="sq")
                    for _ in range(2):
                        nc.vector.tensor_tensor(out=sq, in0=y, in1=y, op=ALU.mult)
                        nc.vector.tensor_tensor(out=sq, in0=sq, in1=veps, op=ALU.mult)
                        nc.vector.tensor_scalar(
                            out=sq, in0=sq, scalar1=-0.5, scalar2=1.5,
                            op0=ALU.mult, op1=ALU.add,
                        )
                        nc.vector.tensor_tensor(out=y, in0=y, in1=sq, op=ALU.mult)
                    psp_box["y"] = y

                def pool_mm(pr):
                    def op():
                        y = psp_box["y"]
                        if pr == 0:
                            psp_box["psp"] = psp_pool.tile([8, D], F32, tag="psp", name="psp")
                        a_sb = a_tiles[pr]
                        nc.vector.tensor_scalar(
                            out=a_sb, in0=a_sb, scalar1=mv_all[:, pr, 0:1],
                            scalar2=y[:, pr:pr + 1],
                            op0=ALU.subtract, op1=ALU.mult,
                        )
                        nc.tensor.matmul(
                            psp_box["psp"], pw_sb[:, pr * 8:(pr + 1) * 8], a_sb,
                            start=(pr == 0), stop=(pr == 3),
                        )
                    return op

                def finish():
                    out_sb = po_pool.tile([8, D], F32, tag="po")
                    nc.vector.tensor_mul(out_sb, psp_box["psp"], g8)
                    nc.vector.tensor_add(out_sb, out_sb, b8)
                    nc.sync.dma_start(
                        out=out[chunk0:chunk0 + CH, :], in_=out_sb
                    )
                return [fisr] + [pool_mm(pr) for pr in range(4)] + [finish]

            chunk0 = idx * CH
            state["tail"] = make_tail(mv_all, a_tiles, chunk0)
            return ops

        def emit_interleaved(a_ops, b_ops):
            """1 A-op : 2 B-ops round-robin; stragglers appended."""
            ia, ib = 0, 0
            while ia < len(a_ops) or ib < len(b_ops):
                if ia < len(a_ops):
                    a_ops[ia]()
                    ia += 1
                for _ in range(2):
                    if ib < len(b_ops):
                        b_ops[ib]()
                        ib += 1

        with (tc.For_i(0, reps, 1) if reps > 1 else nullcontext()):
            a_ops, prev = stage_a_ops(0)
            for op in a_ops:
                op()
            for idx in range(1, n_sup):
                a_ops, cur = stage_a_ops(idx)
                b_ops = stage_b_ops(idx - 1, *prev)
                emit_interleaved(a_ops, b_ops)
                prev = cur
            for op in stage_b_ops(n_sup - 1, *prev):
                op()
            if state["tail"] is not None:
                for op in state["tail"]:
                    op()
                state["tail"] = None

    nc.finalize()
    return nc


def host_inputs(x, conv_w, conv_b, ln_g, ln_b, n_cores):
    """Shard + transform inputs for the device program."""
    x = np.asarray(x)
    B, T, _ = x.shape
    b_loc = B // n_cores
    n_sup = T // SUP

    x16 = np.asarray(x, dtype=np.float16)
    # chunk-shifted copies (zero at chunk boundaries)
    xc = x16.reshape(B, T // L, L, D)
    xm = np.zeros_like(xc)
    xm[:, :, 1:, :] = xc[:, :, :-1, :]
    xp = np.zeros_like(xc)
    xp[:, :, :-1, :] = xc[:, :, 1:, :]
    xm = xm.reshape(B, T, D)
    xp = xp.reshape(B, T, D)

    def dev_layout(a):
        # [B, T, D] -> [B, n_sup, 128(dp), 4(db), SUP(t)]
        a = a.reshape(B, n_sup, SUP, 4, 128)
        return np.ascontiguousarray(a.transpose(0, 1, 4, 3, 2))

    xq_l = dev_layout(x16)
    xm_l = dev_layout(xm)
    xp_l = dev_layout(xp)

    # wt_sb[p_i, tap*4+ib, o] = conv_w[o, ib*128+p_i, tap]
    w = np.asarray(conv_w, dtype=np.float32)  # [O, I, 2]
    wt = np.empty((128, 8, D), dtype=np.float16)
    for tap in range(2):
        for ib in range(4):
            wt[:, tap * 4 + ib, :] = w[:, ib * 128:(ib + 1) * 128, tap].T

    pw = np.zeros((128, 32), dtype=np.float16)
    for tp in range(128):
        for p in range(4):
            pw[tp, p * 8 + 2 * p + tp // 64] = 1.0 / 64
    ident = np.eye(128, dtype=np.float16)
    common = {
        "wt": wt,
        "cb": np.ascontiguousarray(np.asarray(conv_b, dtype=np.float32)),
        "g": np.ascontiguousarray(np.asarray(ln_g, dtype=np.float32)),
        "bb": np.ascontiguousarray(np.asarray(ln_b, dtype=np.float32)),
        "pw": pw,
        "ident": ident,
    }
    in_maps = []
    for c in range(n_cores):
        m = dict(common)
        m["xq"] = np.ascontiguousarray(xq_l[c * b_loc:(c + 1) * b_loc])
        m["xm"] = np.ascontiguousarray(xm_l[c * b_loc:(c + 1) * b_loc])
        m["xp"] = np.ascontiguousarray(xp_l[c * b_loc:(c + 1) * b_loc])
        in_maps.append(m)
    return in_maps


def kernel(x, conv_w, conv_b, ln_g, ln_b, n_stages):
    x = np.asarray(x)
    B, T, d = x.shape
    assert d == D and int(n_stages) * L == T, (x.shape, n_stages)
    n_cores = 8
    nc = build_program(B // n_cores, T, reps=1)
    in_maps = host_inputs(x, conv_w, conv_b, ln_g, ln_b, n_cores)
    res = run_bass_kernel_spmd(nc, in_maps, list(range(n_cores)))
    outs = [res.results[c]["out"] for c in range(n_cores)]
    full = np.concatenate(outs, axis=0)  # [B*n_stages, D]
    return full.reshape(B, int(n_stages), D).astype(np.float32)


if __name__ == "__main__":
    rng = np.random.default_rng(0)
    x = rng.standard_normal((32, 4096, D), dtype=np.float32)
    conv_w = (rng.standard_normal((D, D, 2)) / np.sqrt(2 * D)).astype(np.float32)
    conv_b = (rng.standard_normal(D) * 0.02).astype(np.float32)
    out = kernel(x, conv_w, conv_b, np.ones(D, np.float32), np.zeros(D, np.float32), 64)
    print(out.shape, out.dtype)


# revision 14
# speedup vs baseline: 1.7869x; 1.7869x over previous
"""Trainium2 Bass kernel for nn_ChunkedConvblock (chunked conv + local attention + LN + pool).

Reference computation per chunk of L=64 timesteps (D=512):
  ct = relu(conv1d(x^T, k=2, dilation=2, pad=1) + cb)^T     # [L, D]
  scores = (x @ ct^T) / sqrt(D); p = softmax(scores, -1)
  h = LN(p @ ct + ct) * g + b ; out = mean_t(h)             # [D]

Distribution: data-parallel over batch B=32 across 8 cores (4 rows/core).
Each core processes 256 chunks as 32 "supertiles" of 8 chunks (512 steps).

v6 strategy:
  - host ships x pre-transposed (d on partitions) AND pre-shifted for the
    two conv taps (xm = x[t-1], xp = x[t+1], zeroed at chunk bounds), fp16,
    laid out so each supertile slice is one contiguous 512KB DMA.
  - 2-stage software pipeline with FINE-GRAINED INTERLEAVED EMISSION:
    the attention block of supertile s is emitted matmul-by-matmul between
    the conv matmuls of supertile s+1, so every LDWEIGHTS-bound N=128
    matmul (ct transposes, scores, p-transposes) hides its weight load
    under a 213ns stream-bound conv matmul.
  - ACT uses only {Relu, Exp, Copy} = one activation table set -> no
    1.28us table reloads. rstd = fast-inverse-sqrt (bit trick + 2 Newton
    steps) on DVE ints.
  - layernorm gain/bias commute with the time-mean, applied after pooling.
"""

import sys

if "/opt/trn_rl_repo" not in sys.path:
    sys.path.insert(0, "/opt/trn_rl_repo")

import numpy as np

import concourse.bass as bass
from concourse.bacc import Bacc
from concourse import mybir
from concourse.tile import TileContext
from concourse.bass_utils import run_bass_kernel_spmd

F32 = mybir.dt.float32
F16 = mybir.dt.float16
I32 = mybir.dt.int32
AF = mybir.ActivationFunctionType
ALU = mybir.AluOpType

D = 512
L = 64
SUP = 512          # timesteps per supertile
CH = SUP // L      # 8 chunks per supertile
FISR_MAGIC = 0x5F3759DF


def build_program(B_loc: int, T: int, reps: int = 1):
    """Build the per-core Bass program."""
    n_sup_row = T // SUP
    n_sup = B_loc * n_sup_row
    n_chunks = B_loc * T // L

    nc = Bacc()
    xq_d = nc.declare_dram_parameter("xq", [B_loc, n_sup_row, 128, 4, SUP], F16, isOutput=False)
    xm_d = nc.declare_dram_parameter("xm", [B_loc, n_sup_row, 128, 4, SUP], F16, isOutput=False)
    xp_d = nc.declare_dram_parameter("xp", [B_loc, n_sup_row, 128, 4, SUP], F16, isOutput=False)
    wt = nc.declare_dram_parameter("wt", [128, 8, D], F16, isOutput=False)
    cb = nc.declare_dram_parameter("cb", [D], F32, isOutput=False)
    g = nc.declare_dram_parameter("g", [D], F32, isOutput=False)
    bb = nc.declare_dram_parameter("bb", [D], F32, isOutput=False)
    pw = nc.declare_dram_parameter("pw", [128, 32], F16, isOutput=False)
    ident = nc.declare_dram_parameter("ident", [128, 128], F16, isOutput=False)
    out = nc.declare_dram_parameter("out", [n_chunks, D], F32, isOutput=True)

    inv_sqrt_d = float(1.0 / np.sqrt(D))
    from contextlib import nullcontext

    with TileContext(nc) as tc, \
         tc.tile_pool(name="singles", bufs=1) as singles, \
         tc.tile_pool(name="xq", bufs=4) as xq_pool, \
         tc.tile_pool(name="xm", bufs=4) as xm_pool, \
         tc.tile_pool(name="xp", bufs=4) as xp_pool, \
         tc.tile_pool(name="ctT", bufs=4) as ctT_pool, \
         tc.tile_pool(name="ctn", bufs=3) as ctn_pool, \
         tc.tile_pool(name="soft", bufs=6) as soft_pool, \
         tc.tile_pool(name="acc", bufs=6) as acc_pool, \
         tc.tile_pool(name="stat", bufs=6) as stat_pool, \
         tc.tile_pool(name="po", bufs=2) as po_pool, \
         tc.tile_pool(name="psc", bufs=2, space="PSUM") as psc_pool, \
         tc.tile_pool(name="ps", bufs=5, space="PSUM") as ps_pool, \
         tc.tile_pool(name="psp", bufs=1, space="PSUM") as psp_pool:

        # ---- one-time constants ----
        wt_sb = singles.tile([128, 8, D], F16)
        nc.sync.dma_start(out=wt_sb, in_=wt[:, :, :])
        ident_sb = singles.tile([128, 128], F16)
        nc.sync.dma_start(out=ident_sb, in_=ident[:, :])
        cb_sb = singles.tile([128, 4], F32)
        nc.sync.dma_start(out=cb_sb, in_=cb.rearrange("(j p) -> p j", p=128))
        pw_sb = singles.tile([128, 32], F16)
        nc.sync.dma_start(out=pw_sb, in_=pw[:, :])
        g_ap = g[:]
        b_ap = bb[:]
        g8 = singles.tile([8, D], F32)
        nc.sync.dma_start(
            out=g8,
            in_=bass.AP(tensor=g_ap.tensor, offset=g_ap.offset,
                        ap=[[0, 8]] + list(g_ap.ap)),
        )
        b8 = singles.tile([8, D], F32)
        nc.sync.dma_start(
            out=b8,
            in_=bass.AP(tensor=b_ap.tensor, offset=b_ap.offset,
                        ap=[[0, 8]] + list(b_ap.ap)),
        )
        # persistent softmax tiles: off-diagonal (cross-chunk) blocks are
        # zeroed once; the per-supertile exps only rewrite the diagonals
        p_pers = []
        for i in range(4):
            p_sb = singles.tile([128, 128], F16, tag=f"p{i}")
            nc.vector.memset(p_sb[0:64, 64:128], 0.0)
            nc.vector.memset(p_sb[64:128, 0:64], 0.0)
            p_pers.append(p_sb)

        def stage_a_ops(idx):
            """Closure list: loads + conv MMs (relu attached to each 8th)."""
            row, s = divmod(idx, n_sup_row)
            xq = xq_pool.tile([128, 4, SUP], F16, tag="xq")
            xm = xm_pool.tile([128, 4, SUP], F16, tag="xm")
            xp = xp_pool.tile([128, 4, SUP], F16, tag="xp")
            ctT = ctT_pool.tile([128, 4, SUP], F16, tag="ctT")
            ops = []

            def loads():
                nc.sync.dma_start(out=xq, in_=xq_d[row, s])
                nc.sync.dma_start(out=xm, in_=xm_d[row, s])
                nc.sync.dma_start(out=xp, in_=xp_d[row, s])
            ops.append(loads)

            psc_box = {}

            def conv_mm(ob, k):
                def op():
                    if k == 0:
                        psc_box[ob] = psc_pool.tile([128, SUP], F32, tag="psc", name="psc")
                    tap, ib = divmod(k, 4)
                    xtap = xm if tap == 0 else xp
                    nc.tensor.matmul(
                        psc_box[ob],
                        wt_sb[:, tap * 4 + ib, ob * 128:(ob + 1) * 128],
                        xtap[:, ib, :],
                        start=(k == 0),
                        stop=(k == 7),
                    )
                    if k == 7:
                        nc.scalar.activation(
                            out=ctT[:, ob, :], in_=psc_box[ob], func=AF.Relu,
                            bias=cb_sb[:, ob:ob + 1], scale=1.0,
                        )
                return op

            for ob in range(4):
                for k in range(8):
                    ops.append(conv_mm(ob, k))
            return ops, (xq, ctT)

        state = {"tail": None}

        def stage_b_ops(idx, xq, ctT):
            """Closure list: prev tail, ct transposes, attention block."""
            ops = []
            if state["tail"] is not None:
                ops.extend(state["tail"])
                state["tail"] = None

            ctn = ctn_pool.tile([128, 4, D], F16, tag="ctn")
            ps2_box = {}

            def transpose_op(tb, db):
                def op():
                    if db == 0:
                        ps2_box[tb] = ps_pool.tile([128, D], F16, tag="ps", name="ps2")
                    nc.tensor.transpose(
                        ps2_box[tb][:, db * 128:(db + 1) * 128],
                        ctT[:, db, tb * 128:(tb + 1) * 128],
                        ident_sb,
                    )
                    if db == 3:
                        if tb % 2 == 0:
                            nc.scalar.copy(out=ctn[:, tb, :], in_=ps2_box[tb])
                        else:
                            nc.vector.tensor_copy(
                                out=ctn[:, tb, :], in_=ps2_box[tb])
                return op

            den_all = stat_pool.tile([128, 4], F32, tag="den")
            rec_all = stat_pool.tile([128, 4], F32, tag="rec")
            mv_all = stat_pool.tile([128, 4, 2], F32, tag="mv")
            pT_tiles = [None] * 4
            a_tiles = [None] * 4
            pss_box = {}

            def scores_mm(pr, db):
                def op():
                    if db == 0:
                        pss_box[pr] = ps_pool.tile([128, 128], F32, tag="ps", name="pss")
                    nc.tensor.matmul(
                        pss_box[pr],
                        xq[:, db, pr * 128:(pr + 1) * 128],
                        ctT[:, db, pr * 128:(pr + 1) * 128],
                        start=(db == 0), stop=(db == 3),
                    )
                    if db == 3:
                        p_sb = p_pers[pr]
                        nc.scalar.activation(
                            out=p_sb[0:64, 0:64], in_=pss_box[pr][0:64, 0:64],
                            func=AF.Exp, scale=inv_sqrt_d,
                            accum_out=den_all[0:64, pr:pr + 1],
                        )
                        nc.scalar.activation(
                            out=p_sb[64:128, 64:128],
                            in_=pss_box[pr][64:128, 64:128],
                            func=AF.Exp, scale=inv_sqrt_d,
                            accum_out=den_all[64:128, pr:pr + 1],
                        )
                        nc.vector.reciprocal(
                            rec_all[:, pr:pr + 1], den_all[:, pr:pr + 1]
                        )
                return op

            def p_transpose(pr):
                def op():
                    ps3 = ps_pool.tile([128, 128], F16, tag="ps")
                    nc.tensor.transpose(ps3, p_pers[pr], ident_sb)
                    pT_sb = soft_pool.tile([128, 128], F16, tag="pT")
                    nc.scalar.copy(out=pT_sb, in_=ps3)
                    pT_tiles[pr] = pT_sb
                return op

            def attn_mm(pr):
                def op():
                    psa = ps_pool.tile([128, D], F32, tag="ps")
                    nc.tensor.matmul(psa, pT_tiles[pr], ctn[:, pr, :])
                    a_sb = acc_pool.tile([128, D], F16, tag="a")
                    nc.vector.scalar_tensor_tensor(
                        out=a_sb, in0=psa, scalar=rec_all[:, pr:pr + 1],
                        in1=ctn[:, pr, :], op0=ALU.mult, op1=ALU.add,
                    )
                    st = stat_pool.tile([128, 6], F32, tag="st")
                    nc.vector.bn_stats(st, a_sb)
                    nc.vector.bn_aggr(mv_all[:, pr, :], st)
                    a_tiles[pr] = a_sb
                return op

            # transposes first (ctn needed by attn), then the pair pipeline
            for tb in range(4):
                for db in range(4):
                    ops.append(transpose_op(tb, db))
            sched = []
            for step in range(6):
                if step < 4:
                    sched.extend(scores_mm(step, db) for db in range(4))
                if 1 <= step <= 4:
                    sched.append(p_transpose(step - 1))
                if step >= 2:
                    sched.append(attn_mm(step - 2))
            ops.extend(sched)

            def make_tail(mv_all, a_tiles, chunk0):
                psp_box = {}

                def fisr():
                    # rstd = 1/sqrt(var + eps) via FISR + 2 Newton steps
                    veps = stat_pool.tile([128, 4], F32, tag="veps")
                    nc.vector.tensor_scalar(
                        out=veps, in0=mv_all[:, :, 1], scalar1=1e-5,
                        scalar2=None, op0=ALU.add,
                    )
                    y = stat_pool.tile([128, 4], F32, tag="y")
                    yi = y.bitcast(I32)
                    nc.vector.tensor_scalar(
                        out=yi, in0=veps.bitcast(I32), scalar1=1,
                        scalar2=None, op0=ALU.arith_shift_right,
                    )
                    nc.vector.tensor_scalar(
                        out=yi, in0=yi, scalar1=-1, scalar2=FISR_MAGIC,
                        op0=ALU.mult, op1=ALU.add,
                    )
                    sq = stat_pool.tile([128, 4], F32, tag="sq")
                    for _ in range(2):
                        nc.vector.tensor_tensor(out=sq, in0=y, in1=y, op=ALU.mult)
                        nc.vector.tensor_tensor(out=sq, in0=sq, in1=veps, op=ALU.mult)
                        nc.vector.tensor_scalar(
                            out=sq, in0=sq, scalar1=-0.5, scalar2=1.5,
                            op0=ALU.mult, op1=ALU.add,
                        )
                        nc.vector.tensor_tensor(out=y, in0=y, in1=sq, op=ALU.mult)
                    psp_box["y"] = y

                def pool_mm(pr):
                    def op():
                        y = psp_box["y"]
                        if pr == 0:
                            psp_box["psp"] = psp_pool.tile([8, D], F32, tag="psp", name="psp")
                        a_sb = a_tiles[pr]
                        nc.vector.tensor_scalar(
                            out=a_sb, in0=a_sb, scalar1=mv_all[:, pr, 0:1],
                            scalar2=y[:, pr:pr + 1],
                            op0=ALU.subtract, op1=ALU.mult,
                        )
                        nc.tensor.matmul(
                            psp_box["psp"], pw_sb[:, pr * 8:(pr + 1) * 8], a_sb,
                            start=(pr == 0), stop=(pr == 3),
                        )
                    return op

                def finish():
                    out_sb = po_pool.tile([8, D], F32, tag="po")
                    nc.vector.tensor_mul(out_sb, psp_box["psp"], g8)
                    nc.vector.tensor_add(out_sb, out_sb, b8)
                    nc.sync.dma_start(
                        out=out[chunk0:chunk0 + CH, :], in_=out_sb
                    )
                return [fisr] + [pool_mm(pr) for pr in range(4)] + [finish]

            chunk0 = idx * CH
            state["tail"] = make_tail(mv_all, a_tiles, chunk0)
            return ops

        def emit_interleaved(a_ops, b_ops):
            """1 A-op : 2 B-ops round-robin; stragglers appended."""
            ia, ib = 0, 0
            while ia < len(a_ops) or ib < len(b_ops):
                if ia < len(a_ops):
                    a_ops[ia]()
                    ia += 1
                for _ in range(2):
                    if ib < len(b_ops):
                        b_ops[ib]()
                        ib += 1

        with (tc.For_i(0, reps, 1) if reps > 1 else nullcontext()):
            a_ops, prev = stage_a_ops(0)
            for op in a_ops:
                op()
            for idx in range(1, n_sup):
                a_ops, cur = stage_a_ops(idx)
                b_ops = stage_b_ops(idx - 1, *prev)
                emit_interleaved(a_ops, b_ops)
                prev = cur
            for op in stage_b_ops(n_sup - 1, *prev):
                op()
            if state["tail"] is not None:
                for op in state["tail"]:
                    op()
                state["tail"] = None

    nc.finalize()
    return nc


def host_inputs(x, conv_w, conv_b, ln_g, ln_b, n_cores):
    """Shard + transform inputs for the device program."""
    x = np.asarray(x)
    B, T, _ = x.shape
    b_loc = B // n_cores
    n_sup = T // SUP

    x16 = np.asarray(x, dtype=np.float16)
    # chunk-shifted copies (zero at chunk boundaries)
    xc = x16.reshape(B, T // L, L, D)
    xm = np.zeros_like(xc)
    xm[:, :, 1:, :] = xc[:, :, :-1, :]
    xp = np.zeros_like(xc)
    xp[:, :, :-1, :] = xc[:, :, 1:, :]
    xm = xm.reshape(B, T, D)
    xp = xp.reshape(B, T, D)

    def dev_layout(a):
        # [B, T, D] -> [B, n_sup, 128(dp), 4(db), SUP(t)]
        a = a.reshape(B, n_sup, SUP, 4, 128)
        return np.ascontiguousarray(a.transpose(0, 1, 4, 3, 2))

    xq_l = dev_layout(x16)
    xm_l = dev_layout(xm)
    xp_l = dev_layout(xp)

    # wt_sb[p_i, tap*4+ib, o] = conv_w[o, ib*128+p_i, tap]
    w = np.asarray(conv_w, dtype=np.float32)  # [O, I, 2]
    wt = np.empty((128, 8, D), dtype=np.float16)
    for tap in range(2):
        for ib in range(4):
            wt[:, tap * 4 + ib, :] = w[:, ib * 128:(ib + 1) * 128, tap].T

    pw = np.zeros((128, 32), dtype=np.float16)
    for tp in range(128):
        for p in range(4):
            pw[tp, p * 8 + 2 * p + tp // 64] = 1.0 / 64
    ident = np.eye(128, dtype=np.float16)
    common = {
        "wt": wt,
        "cb": np.ascontiguousarray(np.asarray(conv_b, dtype=np.float32)),
        "g": np.ascontiguousarray(np.asarray(ln_g, dtype=np.float32)),
        "bb": np.ascontiguousarray(np.asarray(ln_b, dtype=np.float32)),
        "pw": pw,
        "ident": ident,
    }
    in_maps = []
    for c in range(n_cores):
        m = dict(common)
        m["xq"] = np.ascontiguousarray(xq_l[c * b_loc:(c + 1) * b_loc])
        m["xm"] = np.ascontiguousarray(xm_l[c * b_loc:(c + 1) * b_loc])
        m["xp"] = np.ascontiguousarray(xp_l[c * b_loc:(c + 1) * b_loc])
        in_maps.append(m)
    return in_maps


def kernel(x, conv_w, conv_b, ln_g, ln_b, n_stages):
    x = np.asarray(x)
    B, T, d = x.shape
    assert d == D and int(n_stages) * L == T, (x.shape, n_stages)
    n_cores = 8
    nc = build_program(B // n_cores, T, reps=1)
    in_maps = host_inputs(x, conv_w, conv_b, ln_g, ln_b, n_cores)
    res = run_bass_kernel_spmd(nc, in_maps, list(range(n_cores)))
    outs = [res.results[c]["out"] for c in range(n_cores)]
    full = np.concatenate(outs, axis=0)  # [B*n_stages, D]
    return full.reshape(B, int(n_stages), D).astype(np.float32)


if __name__ == "__main__":
    rng = np.random.default_rng(0)
    x = rng.standard_normal((32, 4096, D), dtype=np.float32)
    conv_w = (rng.standard_normal((D, D, 2)) / np.sqrt(2 * D)).astype(np.float32)
    conv_b = (rng.standard_normal(D) * 0.02).astype(np.float32)
    out = kernel(x, conv_w, conv_b, np.ones(D, np.float32), np.zeros(D, np.float32), 64)
    print(out.shape, out.dtype)
